# revision 39
# baseline (speedup 1.0000x reference)
"""Conv4d (B=2, Ci=32, Co=64, 16^4 spatial, k=3^4, stride 1, pad 1) on 8
Trainium2 NeuronCores.

Sharding: 8 cores = batch(2) x T-quarters(4). Each core computes
out[64co, 4t, 16d, 16h, 16w] for its (b, t-quarter).

v2 design (vs baseline):
- bf16 x/weights (fp32 PSUM accumulate): halves DMA-in, enables
  standalone-LDWEIGHTS weight reuse (fp32r forbids it).
- Full 128x128 PE array: 4 row groups (D-quarters, K=32ci each) x 2 col
  groups (d-pair within quarter, M=64co at cols 0-63 / 64-127). 8
  concurrent sub-matmuls per tap via tile_position=(32r, 64c).
- Weight reuse: per tap+position one LDWEIGHTS feeds 2 matmuls (the two
  output frames of the phase). Redundant LDWEIGHTS are deleted from the
  legalized module before compile ("surgery").
- 2 phases of 2 output-T frames: PSUM = 8 banks of [128, 512] fp32
  (partitions 0-63 <- col group 0 = d-pair 0, 64-127 <- d-pair 1).
- PE warm-up matmuls at t=0 (no data deps) keep the PE HAM busy during
  the input DMA so real matmuls run at 2.4 GHz from the start.
"""
import sys

sys.path.insert(0, "/opt/trn_rl_repo")
import numpy as np
import ml_dtypes

N_CORES = 8
NWARM = 12
TAPS = [(kt, kd, kh, kw) for kt in range(3) for kd in range(3)
        for kh in range(3) for kw in range(3)]

_NC = None


def _dedupe_ldweights(nc, covered_memref_hint="w2"):
    """Remove redundant InstLdweights:
    1) exact repeats: identical weights AP at the same tile position with
       no intervening load at that position;
    2) 64-col loads from the duplicated-weights tensor (name contains
       `covered_memref_hint`): those are always covered by the preceding
       explicit 128-col ldweights of the same row band.
    Waits/deps of removed loads merge into the following instruction."""
    from concourse import mybir

    removed_total = 0
    for blk in nc.main_func.blocks:
        insts = list(blk.instructions)
        last = {}
        keep = []
        pending = None  # removed LDW whose waits must move to next inst
        for inst in insts:
            if pending is not None:
                # merge removed LDW's sync deps + waits into this inst
                try:
                    inst.merge_dependencies_from(pending)
                except Exception:
                    pass
                psi = pending.sync_info
                if psi is not None and (psi.on_wait or psi.on_update):
                    si = inst.sync_info
                    if si is None:
                        inst.sync_info = mybir.SyncInfo(
                            on_wait=list(psi.on_wait),
                            on_update=list(psi.on_update))
                    else:
                        inst.sync_info = mybir.SyncInfo(
                            on_wait=list(si.on_wait) + list(psi.on_wait),
                            on_update=list(si.on_update) + list(psi.on_update))
                pending = None
            if isinstance(inst, mybir.InstLdweights):
                ap = inst.ins[0]
                dims = [tuple(p) for p in ap.ap]
                free = 1
                for _, sz in dims[1:]:
                    free *= sz
                tp = tuple(inst.tile_position) if inst.tile_position else (0, 0)
                key = str(ap)
                if last.get(tp) == key:
                    pending = inst
                    removed_total += 1
                    continue
                last[tp] = key
            keep.append(inst)
        if len(keep) != len(insts):
            blk.instructions = keep
    return removed_total


def _build():
    global _NC
    if _NC is not None:
        return _NC
    import concourse.bacc as bacc
    import concourse.tile as tile
    from concourse import mybir

    f32 = mybir.dt.float32
    bf16 = mybir.dt.bfloat16

    nc = bacc.Bacc("TRN2", debug=False, target_bir_lowering=False,
                   num_devices=N_CORES)
    xq = nc.dram_tensor("xq", [128, 11664], bf16, kind="ExternalInput")
    wq = nc.dram_tensor("wq", [128, 5184], bf16, kind="ExternalInput")
    bq = nc.dram_tensor("biasq", [128, 1], f32, kind="ExternalInput")
    # out layout: [128, 8192] where partition p = 64*c + co (c = d-pair
    # half), cols = pair_k(8: phase*4+r) x to_half(2) x (dd2, hw256).
    # Host reassembles. 4 KB contiguous DRAM rows per pair-DMA.
    out = nc.dram_tensor("out", [128, 8192], mybir.dt.float16, kind="ExternalOutput")

    with tile.TileContext(nc) as tc:
        with tc.tile_pool(name="xp", bufs=1) as xp, \
             tc.tile_pool(name="wp", bufs=1) as wp, \
             tc.tile_pool(name="op", bufs=6) as op_, \
             tc.tile_pool(name="pp", bufs=8, space="PSUM") as pp:
            # --- PE warm-up: full-array MMs (HAM needs high aggregate
            # PE activity), no data deps, runs during input DMA ---
            wu_w = wp.tile([128, 128], bf16)
            wu_x = wp.tile([128, 512], bf16)
            nc.vector.memset(wu_w[:], 0.0)
            nc.vector.memset(wu_x[:], 0.0)
            wu_ps = pp.tile([128, 512], f32, tag="ps", name="wu_ps")
            for _ in range(NWARM):
                nc.tensor.matmul(wu_ps[:], wu_w[:], wu_x[:],
                                 start=True, stop=True, tile_position=(0, 0))

            # --- input DMAs, ordered so first-tap deps arrive first ---
            wtile = wp.tile([128, 5184], bf16, name="w2")
            btile = wp.tile([128, 1], f32)
            xtile = xp.tile([128, 11664], bf16)

            def xdma(tf):
                nc.sync.dma_start(xtile[:, tf * 1944:(tf + 1) * 1944],
                                  xq.ap()[:, tf * 1944:(tf + 1) * 1944])

            def wdma(c0, c1):
                nc.gpsimd.dma_start(wtile[:, c0:c1], wq.ap()[:, c0:c1])

            xdma(0)
            wdma(0, 576)         # taps 0-8 (kt=0, kd=0)
            xdma(1)
            nc.sync.dma_start(btile[:], bq.ap()[:])
            xdma(2)
            wdma(576, 1728)      # rest of kt=0
            xdma(3)
            wdma(1728, 5184)
            xdma(4)
            xdma(5)

            xv = xtile.rearrange("p (t d h w) -> p t d h w",
                                 t=6, d=6, h=18, w=18)

            for phase, tos in enumerate(((0, 1), (2, 3))):
                ps = {}
                for to in tos:
                    for r in range(4):
                        ps[(to, r)] = pp.tile([128, 512], f32, tag="ps",
                                              name=f"ps_{to}_{r}")
                for i, (kt, kd, kh, kw) in enumerate(TAPS):
                    for r in range(4):
                        lhsT = wtile[32 * r:32 * r + 32,
                                     i * 64:(i + 1) * 64]
                        for to in tos:
                            for c in range(2):
                                rhs = xv[32 * r:32 * r + 32, to + kt,
                                         2 * c + kd: 2 * c + kd + 2,
                                         kh:kh + 16, kw:kw + 16]
                                nc.tensor.matmul(
                                    ps[(to, r)][64 * c:64 * c + 64, :],
                                    lhsT, rhs,
                                    start=(i == 0), stop=(i == 80),
                                    tile_position=(32 * r, 64 * c))
                # epilogue: banks must release in the order phase B's
                # tap-0 matmuls consume them: (to_lo,r0),(to_hi,r0),
                # (to_lo,r1),... -> r-major, vector/scalar in parallel.
                for r in range(4):
                    stg = op_.tile([128, 1024], mybir.dt.float16, tag="ob",
                                   name=f"o_{phase}_{r}")
                    for ti, to in enumerate(tos):
                        pst = ps[(to, r)]
                        dst = stg[:, ti * 512:(ti + 1) * 512]
                        if ti == 0:
                            nc.vector.tensor_scalar_add(dst, pst[:],
                                                        btile[:, 0:1])
                        else:
                            nc.scalar.activation(
                                dst, pst[:],
                                mybir.ActivationFunctionType.Identity,
                                bias=btile[:, 0:1])
                    k = phase * 4 + r
                    engs = (nc.sync, nc.gpsimd, nc.sync, nc.gpsimd)
                    for h, eng in enumerate(engs):
                        eng.dma_start(
                            out.ap()[32 * h:32 * h + 32,
                                     k * 1024:(k + 1) * 1024],
                            stg[32 * h:32 * h + 32, :])
    _dedupe_ldweights(nc)
    nc.compile()
    _NC = nc
    return nc


def _prep_inputs(x, weight, bias):
    x = np.asarray(x, dtype=np.float32)
    weight = np.asarray(weight, dtype=np.float32)
    bias = np.asarray(bias, dtype=np.float32)
    bf16 = ml_dtypes.bfloat16

    w9 = weight.reshape(64, 32, 81).transpose(2, 1, 0)  # [tap, ci, co]
    warr = np.ascontiguousarray(w9.transpose(1, 0, 2)).reshape(32, 81 * 64)
    wq = np.tile(warr, (4, 1)).astype(bf16)  # [128, 5184]
    bq = np.concatenate([bias, bias]).reshape(128, 1).astype(np.float32)

    in_maps = []
    for b in range(2):
        xpad = np.pad(x[b], ((0, 0), (1, 1), (1, 1), (1, 1), (1, 1)))
        for tq in range(4):
            xt = xpad[:, 4 * tq:4 * tq + 6]  # [32, 6, 18, 18, 18]
            xqc = np.empty((128, 11664), np.float32)
            for r in range(4):
                xqc[32 * r:32 * r + 32] = \
                    xt[:, :, 4 * r:4 * r + 6].reshape(32, -1)
            in_maps.append({"xq": xqc.astype(bf16), "wq": wq, "biasq": bq})
    return in_maps


def run_spmd(x, weight, bias, trace=False, trace_cores=None, tmpdir=None):
    """Returns (output ndarray, BassKernelResults)."""
    from concourse.bass_utils import run_bass_kernel_spmd
    nc = _build()
    in_maps = _prep_inputs(x, weight, bias)
    res = run_bass_kernel_spmd(nc, in_maps, core_ids=list(range(N_CORES)),
                               trace=trace, trace_cores=trace_cores,
                               tmpdir=tmpdir)
    out = np.empty((2, 64, 16, 16, 16, 16), np.float32)
    for c in range(N_CORES):
        b, tq = c // 4, c % 4
        # [128, 8192]: dims (c2, co64) x (ph2, r4, to2, dd2, hw256);
        # d = 4r + 2c + dd, global to = 2*ph + to
        oc = res.results[c]["out"].astype(np.float32)
        oc = oc.reshape(2, 64, 2, 4, 2, 2, 256)
        oc = oc.transpose(1, 2, 4, 3, 0, 5, 6).reshape(64, 4, 16, 16, 16)
        out[b, :, 4 * tq:4 * tq + 4] = oc
    return out, res


def kernel(x, weight, bias):
    out, _ = run_spmd(x, weight, bias)
    return out


# revision 42
# speedup vs baseline: 1.0232x; 1.0232x over previous
"""Conv4d (B=2, Ci=32, Co=64, 16^4 spatial, k=3^4, stride 1, pad 1) on 8
Trainium2 NeuronCores.

Sharding: 8 cores = batch(2) x T-quarters(4). Each core computes
out[64co, 4t, 16d, 16h, 16w] for its (b, t-quarter).

Design:
- bf16 x/weights (fp32 PSUM accumulate): halves DMA-in and permits
  non-self-loading matmuls (weight reuse), which fp32r forbids.
- Full 128x128 PE array: 4 row groups (D-quarters, K=32ci each) x 2 col
  groups (d-pair within quarter, M=64co at cols 0-63 / 64-127). 8
  concurrent sub-matmuls per tap via tile_position=(32r, 64c).
- Weight reuse: per tap+position one LDWEIGHTS feeds the 2 matmuls (the
  two output frames of the phase). Redundant LDWEIGHTS are deleted from
  the legalized module before compile (_dedupe_ldweights). The weight
  port (~1.2 Gcols/s, 8x64 cols/tap) is the steady-state roofline.
- 2 phases of 2 output-T frames: PSUM = 8 banks of [128, 512] fp32
  (partitions 0-63 <- col group 0 = d-pair 0, 64-127 <- d-pair 1).
  Phase-A epilogues are ordered r-major, vector/scalar paired, so banks
  release in exactly the order phase B's first-tap matmuls claim them.
- PE warm-up matmuls at t=0 (no data deps, full-array so the HAM
  activity monitor sees them) bridge the input DMA so real matmuls run
  at 2.4 GHz from the first tap.
- fp16 output staging/DMA (error << the 2e-2 gate), host converts back.
"""
import sys

sys.path.insert(0, "/opt/trn_rl_repo")
import numpy as np
import ml_dtypes

N_CORES = 8
NWARM = 12
TAPS = [(kt, kd, kh, kw) for kt in range(3) for kd in range(3)
        for kh in range(3) for kw in range(3)]

_NC = None


def _dedupe_ldweights(nc):
    """Remove InstLdweights that reload the identical weights AP at the
    same tile position with no intervening load at that position (the
    tile legalizer emits one load per matmul; consecutive matmuls at a
    position that share weights only need the first). Waits/deps of each
    removed load merge into the following instruction (its matmul)."""
    from concourse import mybir

    removed_total = 0
    for blk in nc.main_func.blocks:
        insts = list(blk.instructions)
        last = {}
        keep = []
        pending = None  # removed LDW whose waits must move to next inst
        for inst in insts:
            if pending is not None:
                # merge removed LDW's sync deps + waits into this inst
                try:
                    inst.merge_dependencies_from(pending)
                except Exception:
                    pass
                psi = pending.sync_info
                if psi is not None and (psi.on_wait or psi.on_update):
                    si = inst.sync_info
                    if si is None:
                        inst.sync_info = mybir.SyncInfo(
                            on_wait=list(psi.on_wait),
                            on_update=list(psi.on_update))
                    else:
                        inst.sync_info = mybir.SyncInfo(
                            on_wait=list(si.on_wait) + list(psi.on_wait),
                            on_update=list(si.on_update) + list(psi.on_update))
                pending = None
            if isinstance(inst, mybir.InstLdweights):
                ap = inst.ins[0]
                tp = tuple(inst.tile_position) if inst.tile_position else (0, 0)
                key = str(ap)
                if last.get(tp) == key:
                    pending = inst
                    removed_total += 1
                    continue
                last[tp] = key
            keep.append(inst)
        if len(keep) != len(insts):
            blk.instructions = keep
    return removed_total


def _build():
    global _NC
    if _NC is not None:
        return _NC
    import concourse.bacc as bacc
    import concourse.tile as tile
    from concourse import mybir

    f32 = mybir.dt.float32
    bf16 = mybir.dt.bfloat16

    nc = bacc.Bacc("TRN2", debug=False, target_bir_lowering=False,
                   num_devices=N_CORES)
    xq = nc.dram_tensor("xq", [128, 11664], bf16, kind="ExternalInput")
    wq = nc.dram_tensor("wq", [128, 5184], bf16, kind="ExternalInput")
    bq = nc.dram_tensor("biasq", [128, 1], f32, kind="ExternalInput")
    # out layout: [128, 8192] where partition p = 64*c + co (c = d-pair
    # half), cols = pair_k(8: phase*4+r) x to_half(2) x (dd2, hw256).
    # Host reassembles. 4 KB contiguous DRAM rows per pair-DMA.
    out = nc.dram_tensor("out", [128, 8192], mybir.dt.float16, kind="ExternalOutput")

    with tile.TileContext(nc) as tc:
        with tc.tile_pool(name="xp", bufs=1) as xp, \
             tc.tile_pool(name="wp", bufs=1) as wp, \
             tc.tile_pool(name="op", bufs=6) as op_, \
             tc.tile_pool(name="pp", bufs=8, space="PSUM") as pp:
            # --- PE warm-up: full-array MMs (HAM needs high aggregate
            # PE activity), no data deps, runs during input DMA ---
            wu_w = wp.tile([128, 128], bf16)
            wu_x = wp.tile([128, 512], bf16)
            nc.vector.memset(wu_w[:], 0.0)
            nc.vector.memset(wu_x[:], 0.0)
            wu_ps = pp.tile([128, 512], f32, tag="ps", name="wu_ps")
            for _ in range(NWARM):
                nc.tensor.matmul(wu_ps[:], wu_w[:], wu_x[:],
                                 start=True, stop=True, tile_position=(0, 0))

            # --- input DMAs, ordered so first-tap deps arrive first ---
            wtile = wp.tile([128, 5184], bf16, name="w2")
            btile = wp.tile([128, 1], f32)
            xtile = xp.tile([128, 11664], bf16)

            def xdma(tf):
                nc.sync.dma_start(xtile[:, tf * 1944:(tf + 1) * 1944],
                                  xq.ap()[:, tf * 1944:(tf + 1) * 1944])

            def wdma(c0, c1):
                nc.gpsimd.dma_start(wtile[:, c0:c1], wq.ap()[:, c0:c1])

            xdma(0)
            wdma(0, 576)         # taps 0-8 (kt=0, kd=0)
            xdma(1)
            nc.sync.dma_start(btile[:], bq.ap()[:])
            xdma(2)
            wdma(576, 1728)      # rest of kt=0
            xdma(3)
            wdma(1728, 5184)
            xdma(4)
            xdma(5)

            xv = xtile.rearrange("p (t d h w) -> p t d h w",
                                 t=6, d=6, h=18, w=18)

            for phase, tos in enumerate(((0, 1), (2, 3))):
                ps = {}
                for to in tos:
                    for r in range(4):
                        ps[(to, r)] = pp.tile([128, 512], f32, tag="ps",
                                              name=f"ps_{to}_{r}")
                for i, (kt, kd, kh, kw) in enumerate(TAPS):
                    for r in range(4):
                        lhsT = wtile[32 * r:32 * r + 32,
                                     i * 64:(i + 1) * 64]
                        for c in range(2):
                            for to in tos:
                                rhs = xv[32 * r:32 * r + 32, to + kt,
                                         2 * c + kd: 2 * c + kd + 2,
                                         kh:kh + 16, kw:kw + 16]
                                nc.tensor.matmul(
                                    ps[(to, r)][64 * c:64 * c + 64, :],
                                    lhsT, rhs,
                                    start=(i == 0), stop=(i == 80),
                                    tile_position=(32 * r, 64 * c))
                # epilogue: banks must release in the order phase B's
                # tap-0 matmuls consume them: (to_lo,r0),(to_hi,r0),
                # (to_lo,r1),... -> r-major, vector/scalar in parallel.
                for r in range(4):
                    stg = op_.tile([128, 1024], mybir.dt.float16, tag="ob",
                                   name=f"o_{phase}_{r}")
                    for ti, to in enumerate(tos):
                        pst = ps[(to, r)]
                        dst = stg[:, ti * 512:(ti + 1) * 512]
                        if ti == 0:
                            nc.vector.tensor_scalar_add(dst, pst[:],
                                                        btile[:, 0:1])
                        else:
                            nc.scalar.activation(
                                dst, pst[:],
                                mybir.ActivationFunctionType.Identity,
                                bias=btile[:, 0:1])
                    k = phase * 4 + r
                    engs = (nc.sync, nc.gpsimd, nc.sync, nc.gpsimd)
                    for h, eng in enumerate(engs):
                        eng.dma_start(
                            out.ap()[32 * h:32 * h + 32,
                                     k * 1024:(k + 1) * 1024],
                            stg[32 * h:32 * h + 32, :])
    _dedupe_ldweights(nc)
    nc.compile()
    _NC = nc
    return nc


def _prep_inputs(x, weight, bias):
    x = np.asarray(x, dtype=np.float32)
    weight = np.asarray(weight, dtype=np.float32)
    bias = np.asarray(bias, dtype=np.float32)
    bf16 = ml_dtypes.bfloat16

    w9 = weight.reshape(64, 32, 81).transpose(2, 1, 0)  # [tap, ci, co]
    warr = np.ascontiguousarray(w9.transpose(1, 0, 2)).reshape(32, 81 * 64)
    wq = np.tile(warr, (4, 1)).astype(bf16)  # [128, 5184]
    bq = np.concatenate([bias, bias]).reshape(128, 1).astype(np.float32)

    in_maps = []
    for b in range(2):
        xpad = np.pad(x[b], ((0, 0), (1, 1), (1, 1), (1, 1), (1, 1)))
        for tq in range(4):
            xt = xpad[:, 4 * tq:4 * tq + 6]  # [32, 6, 18, 18, 18]
            xqc = np.empty((128, 11664), np.float32)
            for r in range(4):
                xqc[32 * r:32 * r + 32] = \
                    xt[:, :, 4 * r:4 * r + 6].reshape(32, -1)
            in_maps.append({"xq": xqc.astype(bf16), "wq": wq, "biasq": bq})
    return in_maps


def run_spmd(x, weight, bias, trace=False, trace_cores=None, tmpdir=None):
    """Returns (output ndarray, BassKernelResults)."""
    from concourse.bass_utils import run_bass_kernel_spmd
    nc = _build()
    in_maps = _prep_inputs(x, weight, bias)
    res = run_bass_kernel_spmd(nc, in_maps, core_ids=list(range(N_CORES)),
                               trace=trace, trace_cores=trace_cores,
                               tmpdir=tmpdir)
    out = np.empty((2, 64, 16, 16, 16, 16), np.float32)
    for c in range(N_CORES):
        b, tq = c // 4, c % 4
        # [128, 8192]: dims (c2, co64) x (ph2, r4, to2, dd2, hw256);
        # d = 4r + 2c + dd, global to = 2*ph + to
        oc = res.results[c]["out"].astype(np.float32)
        oc = oc.reshape(2, 64, 2, 4, 2, 2, 256)
        oc = oc.transpose(1, 2, 4, 3, 0, 5, 6).reshape(64, 4, 16, 16, 16)
        out[b, :, 4 * tq:4 * tq + 4] = oc
    return out, res


def kernel(x, weight, bias):
    out, _ = run_spmd(x, weight, bias)
    return out
